# revision 35
# baseline (speedup 1.0000x reference)
"""CODA-Prompt forward kernel for 8 TRN2 NeuronCores (data-parallel over batch).

Reference computation (forward only; stop_gradient is identity):
    K = (task_count + 1) * 10            # active pool slice, all branches
    x_mean[b,d]  = mean_n x[b,n,d]
    aq[b,k]      = (x_mean . (att[k]*nK[k])) / max(||x_mean*att[k]||, eps)
    P_[b,l,d]    = sum_k aq[b,k] * prompt[k,l,d]
    out          = concat([P_, x], axis=1)            # [B, 8+197, 768]

Device kernel per core (B=32 of 256 batches), built for DMA efficiency:
  x arrives flat+padded [B*197+1, 768].  Each batch is one fully
  sequential in-DMA in token-pair layout [99, 2, 768] (6 KB runs); the
  out-copy writes rows [205b+8, 205b+204) from the same tile plus the
  odd 197th row DRAM->DRAM straight from x, so nothing ever touches the
  P_ rows and there are no DRAM write-write hazards.  The tile's 198th
  row is the next batch's token 0 (garbage); its contribution to the
  token sum is removed with a correction DMA of rows x[b+1, 0, :].

  Token sums accumulate batch-on-partition in PSUM via indicator-
  stationary matmuls (lhsT = e_b x ones built on device from a 4 KB
  flattened-identity constant).  Batches run in 4 groups of 8: as soon
  as a group's sums are in PSUM, its stage 2 (transpose + aq) and
  stage 3 (P_ = aq @ prompt, P_ DMA) run overlapped with the remaining
  streaming, so the serial tail is only the last group's stage 2/3
  instead of all 32 batches'.

Host combines the small pool tensors:
    attnkT[d,k] = att[k,d] * nK[k,d],  attn2T[d,k] = att[k,d]^2,
    prflat[k,:] = prompt[k].reshape(6144)
aq is scale-invariant in x_mean, so the 1/197 mean scaling cancels and
the kernel works with raw token sums.
"""

import numpy as np

TOP_K = 10
LENGTH = 8
EMBED_DIM = 768
N_TOK = 197
B_FULL = 256
N_CORES = 8
B = B_FULL // N_CORES          # 32 batches per core
GB = 8                         # batches per stage-2/3 group
NG = B // GB                   # 4 groups
PF = LENGTH * EMBED_DIM        # 6144 flattened prompt row
XROWS = B * N_TOK + 1          # flat x rows incl one zero pad row
OROWS = B * (LENGTH + N_TOK)   # flat out rows
NP2 = (N_TOK + 1) // 2         # 99 token pairs per batch (last half garbage)

_PROGRAMS = {}


def _build_program(K):
    import concourse.bacc as bacc
    import concourse.mybir as mybir
    import concourse.tile as tile
    from concourse.bass import ts
    from concourse.masks import make_identity
    import concourse.bass as bass

    f32 = mybir.dt.float32
    nc = bacc.Bacc()

    x = nc.dram_tensor("x", [XROWS, EMBED_DIM], f32, kind="ExternalInput")
    prflat = nc.dram_tensor("prflat", [K, PF], f32, kind="ExternalInput")
    attnkT = nc.dram_tensor("attnkT", [EMBED_DIM, K], f32, kind="ExternalInput")
    attn2T = nc.dram_tensor("attn2T", [EMBED_DIM, K], f32, kind="ExternalInput")
    emflat = nc.dram_tensor("emflat", [1, GB * GB], f32, kind="ExternalInput")
    out = nc.dram_tensor("out", [OROWS, EMBED_DIM], f32, kind="ExternalOutput")

    with tile.TileContext(nc) as tc:
        with (
            tc.tile_pool(name="const", bufs=1) as constp,
            tc.tile_pool(name="xt", bufs=14) as xtp,
            tc.tile_pool(name="xs", bufs=8) as xsp,
            tc.tile_pool(name="grp", bufs=2) as grpp,
            tc.tile_pool(name="psg", bufs=2, space="PSUM") as psgp,
            tc.tile_pool(name="pt", bufs=1, space="PSUM") as ptp,
            tc.tile_pool(name="pnq", bufs=1, space="PSUM") as pnqp,
            tc.tile_pool(name="pp", bufs=2, space="PSUM") as ppp,
        ):
            # --- constants (gpsimd queue; big streams go on sync/scalar) ---
            ident = constp.tile([128, 128], f32)
            make_identity(nc, ident)
            # prflat lives at partitions 96..96+K: shifts its load (and the
            # stage-3 reads) onto the otherwise idle high DMA engines
            prflat_sb = constp.tile([128, PF], f32)
            nc.gpsimd.dma_start(out=prflat_sb[64:64 + K], in_=prflat[:, :])
            attnkT_sb = constp.tile([128, 6, K], f32)
            nc.gpsimd.dma_start(
                out=attnkT_sb,
                in_=attnkT[:, :].rearrange("(c p) k -> p c k", p=128))
            attn2T_sb = constp.tile([128, 6, K], f32)
            nc.gpsimd.dma_start(
                out=attn2T_sb,
                in_=attn2T[:, :].rearrange("(c p) k -> p c k", p=128))
            # correction rows: x[b+1, token 0], one tile per group of 8
            # batches (SBUF APs must start at partition 0)
            corr_g = []
            for g in range(NG):
                cg = constp.tile([128, EMBED_DIM], f32, name=f"corr{g}")
                nc.gpsimd.dma_start(out=cg[96:96 + GB], in_=bass.AP(
                    tensor=x[:, :].tensor,
                    offset=(g * GB + 1) * N_TOK * EMBED_DIM,
                    ap=[[N_TOK * EMBED_DIM, GB], [1, EMBED_DIM]]))
                corr_g.append(cg)
            # batch-in-group indicator em[p, bi, c] = (bi == c), replicated
            # over partitions from a 4 KB host constant via a K=1 matmul
            emflat_sb = constp.tile([1, GB * GB], f32)
            nc.gpsimd.dma_start(out=emflat_sb, in_=emflat[:, :])
            onesc = constp.tile([1, 128], f32)
            nc.vector.memset(onesc, 1.0)
            em_sb = constp.tile([128, GB, GB], f32)
            pe0 = ptp.tile([128, GB * GB], f32, tag="pt", name="pt")
            nc.tensor.matmul(pe0, onesc, emflat_sb, start=True, stop=True)
            nc.vector.tensor_copy(em_sb, pe0)

            # Preheat: have PE consume each constant once so no later matmul
            # needs >1 semaphore wait.
            scr = ptp.tile([128, GB * GB], f32, tag="pt", name="pt")
            nc.tensor.matmul(scr[:1, :1], ident[:1, :1], ident[:1, :1],
                             start=True, stop=True)
            nc.tensor.matmul(scr[:1, :1], attnkT_sb[:1, 0, :1],
                             attnkT_sb[:1, 0, :1], start=True, stop=True)
            nc.tensor.matmul(scr[:1, :1], attn2T_sb[:1, 0, :1],
                             attn2T_sb[:1, 0, :1], start=True, stop=True)
            nc.tensor.matmul(scr[:1, :1], prflat_sb[64:65, :1],
                             prflat_sb[64:65, :1], start=True, stop=True)
            nc.tensor.matmul(scr[:1, :1], em_sb[:1, 0, :1], em_sb[:1, 0, :1],
                             start=True, stop=True)

            # Queue plan: ALL in-DMAs on the sync ring so inputs never queue
            # behind the out-copy backlog -- compute then finishes while the
            # out-drain is still running and only P_ tails the window.
            # Outs split scalar/gpsimd by effective rate (HWDGE 1.0 vs
            # SWDGE ~0.56): 20 scalar vs 12 gpsimd.
            in_eng = [nc.sync] * B
            for b in range(5, B, 5):
                if sum(1 for e in in_eng if e is nc.scalar) < 6:
                    in_eng[b] = nc.scalar
            out_eng = [nc.scalar] * B
            for b in range(1, B, 3):
                if sum(1 for e in out_eng if e is nc.gpsimd) < 12:
                    out_eng[b] = nc.gpsimd

            # aq columns for all batches, filled per group as sums complete
            # (based at partition 64 to match prflat for the stage-3 matmul)
            aqT_all = constp.tile([128, B], f32)

            def stage2(g, psh):
                """aq for batches 8g..8g+7, overlapped with streaming."""
                # garbage-row correction on the way out of PSUM
                means = grpp.tile([GB, EMBED_DIM], f32, name="means")
                for h in range(2):
                    nc.vector.tensor_sub(
                        means[:, ts(h, 384)], psh[h],
                        corr_g[g][96:96 + GB, ts(h, 384)])

                meansT = grpp.tile([128, 6, GB], f32, name="meansT")
                for j in range(6):
                    pt = ptp.tile([128, GB * GB], f32, tag="pt", name="pt")
                    nc.tensor.transpose(pt[:, :GB], means[:, ts(j, 128)],
                                        ident[:GB, :GB])
                    if j % 2 == 0:
                        nc.vector.tensor_copy(meansT[:, j, :], pt[:, :GB])
                    else:
                        nc.scalar.copy(meansT[:, j, :], pt[:, :GB])
                sqT = grpp.tile([128, 6, GB], f32, name="sqT")
                nc.vector.tensor_mul(sqT, meansT, meansT)

                pn = pnqp.tile([K, 2, GB], f32, name="pn")
                for j in range(6):
                    nc.tensor.matmul(pn[:, 0, :], attnkT_sb[:, j, :],
                                     meansT[:, j, :],
                                     start=(j == 0), stop=(j == 5))
                for j in range(6):
                    nc.tensor.matmul(pn[:, 1, :], attn2T_sb[:, j, :],
                                     sqT[:, j, :],
                                     start=(j == 0), stop=(j == 5))

                denom = grpp.tile([K, GB], f32, name="denom")
                nc.scalar.sqrt(denom, pn[:, 1, :])
                nc.vector.tensor_scalar_max(denom, denom, 1e-12)
                recip = grpp.tile([K, GB], f32, name="recip")
                nc.vector.reciprocal(recip, denom)
                nc.vector.tensor_mul(aqT_all[64:64 + K, g * GB:(g + 1) * GB],
                                     pn[:, 0, :], recip)

            # every batch's odd 197th row in one strided DRAM->DRAM DMA
            # (SBUF APs can't start at partition 98); rows are untouched by
            # anything else, so it can fire immediately.
            orow = (LENGTH + N_TOK) * EMBED_DIM
            nc.gpsimd.dma_start(
                out=bass.AP(tensor=out[:, :].tensor,
                            offset=(LENGTH + N_TOK - 1) * EMBED_DIM,
                            ap=[[orow, B], [1, EMBED_DIM]]),
                in_=bass.AP(tensor=x[:, :].tensor,
                            offset=(N_TOK - 1) * EMBED_DIM,
                            ap=[[N_TOK * EMBED_DIM, B], [1, EMBED_DIM]]))

            # --- stage 1: stream x, copy to out rows, accumulate sums ------
            # Even batches use SBUF partitions [0, 99); odd batches are
            # split [64, 128) + [0, 35) (the only legal non-zero partition
            # starts are 0/32/64/96) so the high DMA engines -- which only
            # serve partitions 99-127 -- also carry copy traffic.  The DVE
            # fold re-packs odd batches to xs[0:99], so the PE side is
            # identical for all batches.
            psg_tiles = {}
            for b in range(B):
                g, bi = b // GB, b % GB
                r0 = b * N_TOK
                o0 = b * (LENGTH + N_TOK) + LENGTH
                xt = xtp.tile([NP2, 2, EMBED_DIM], f32)
                xs = xsp.tile([NP2, EMBED_DIM], f32)
                in_eng[b].dma_start(
                    out=xt,
                    in_=x[r0:r0 + 2 * NP2, :].rearrange("(p u) d -> p u d",
                                                        u=2))
                out_eng[b].dma_start(
                    out=out[o0:o0 + N_TOK - 1, :].rearrange(
                        "(p u) d -> p u d", u=2),
                    in_=xt[:NP2 - 1])
                nc.vector.tensor_add(xs, xt[:, 0, :], xt[:, 1, :])
                if bi == 0:
                    psg_tiles[g] = [psgp.tile([GB, 384], f32, name=f"psg{h}")
                                    for h in range(2)]
                for h in range(2):
                    nc.tensor.matmul(
                        psg_tiles[g][h],
                        em_sb[:NP2, bi, :], xs[:, ts(h, 384)],
                        start=(bi == 0), stop=(bi == GB - 1))
                if b == B - 1:
                    for w in range(40):
                        nc.tensor.matmul(scr[:, :GB * GB], ident,
                                         ident[:, :GB * GB],
                                         start=True, stop=True)
                if bi == GB - 1:
                    stage2(g, psg_tiles.pop(g))

            # --- stage 3 (single shot, M=32): P_ = aq @ prflat -------------
            # p_sb also lives at partitions 96..128 (idle-engine traffic)
            p_sb = constp.tile([128, PF], f32)
            for h in range(PF // 512):
                pp = ppp.tile([B, 512], f32, tag="pp", name="pp")
                nc.tensor.matmul(pp, aqT_all[64:64 + K, :],
                                 prflat_sb[64:64 + K, ts(h, 512)],
                                 start=True, stop=True)
                if h % 2 == 0:
                    nc.vector.tensor_copy(p_sb[96:128, ts(h, 512)], pp)
                else:
                    nc.scalar.copy(p_sb[96:128, ts(h, 512)], pp)
                if h % 4 == 3:  # P_ written in 3 pipelined strips
                    c0 = (h - 3) * 512
                    nc.sync.dma_start(
                        out=bass.AP(tensor=out[:, :].tensor, offset=c0,
                                    ap=[[orow, B], [1, 2048]]),
                        in_=p_sb[96:128, c0:c0 + 2048])

    nc.finalize()
    return nc


def _host_prep(prompt, attention, prompt_key, task_count):
    K = (int(task_count) + 1) * TOP_K
    pk = np.asarray(prompt_key[:K], dtype=np.float32)
    att = np.asarray(attention[:K], dtype=np.float32)
    pr = np.asarray(prompt[:K], dtype=np.float32)
    nrm = np.sqrt(np.sum(pk * pk, axis=1, keepdims=True, dtype=np.float32))
    nK = pk / np.maximum(nrm, np.float32(1e-12))
    attnkT = np.ascontiguousarray((att * nK).T)
    attn2T = np.ascontiguousarray((att * att).T)
    prflat = np.ascontiguousarray(pr.reshape(K, PF))
    return K, attnkT, attn2T, prflat


def _shard_x(x_embed, i):
    flat = x_embed[i * B:(i + 1) * B].reshape(B * N_TOK, EMBED_DIM)
    pad = np.zeros((1, EMBED_DIM), dtype=np.float32)
    return np.ascontiguousarray(np.concatenate([flat, pad], axis=0))


def _emflat():
    return np.eye(GB, dtype=np.float32).reshape(1, GB * GB)


def kernel(x_embed, prompt, attention, prompt_key, iseval, task_count,
           _want_trace=False, **_trace_kwargs):
    from concourse.bass_utils import run_bass_kernel_spmd

    x_embed = np.asarray(x_embed, dtype=np.float32)
    assert x_embed.shape == (B_FULL, N_TOK, EMBED_DIM)
    K, attnkT, attn2T, prflat = _host_prep(prompt, attention, prompt_key,
                                           task_count)

    if K not in _PROGRAMS:
        _PROGRAMS[K] = _build_program(K)
    nc = _PROGRAMS[K]

    in_maps = []
    for i in range(N_CORES):
        in_maps.append({
            "x": _shard_x(x_embed, i),
            "prflat": prflat,
            "attnkT": attnkT,
            "attn2T": attn2T,
            "emflat": _emflat(),
        })
    res = run_bass_kernel_spmd(nc, in_maps, core_ids=list(range(N_CORES)),
                               trace=_want_trace, **_trace_kwargs)
    full = np.concatenate(
        [res.results[i]["out"].reshape(
            B, LENGTH + N_TOK, EMBED_DIM) for i in range(N_CORES)],
        axis=0)
    if _want_trace:
        return full, res
    return full


# revision 36
# speedup vs baseline: 1.0257x; 1.0257x over previous
"""CODA-Prompt forward kernel for 8 TRN2 NeuronCores (data-parallel over batch).

Reference computation (forward only; stop_gradient is identity):
    K = (task_count + 1) * 10            # active pool slice, all branches
    x_mean[b,d]  = mean_n x[b,n,d]
    aq[b,k]      = (x_mean . (att[k]*nK[k])) / max(||x_mean*att[k]||, eps)
    P_[b,l,d]    = sum_k aq[b,k] * prompt[k,l,d]
    out          = concat([P_, x], axis=1)            # [B, 8+197, 768]

Device kernel per core (B=32 of 256 batches), built for DMA efficiency:
  x arrives flat+padded [B*197+1, 768].  Each batch is one fully
  sequential DMA in token-pair layout [99, 2, 768] (6 KB runs), copied
  back out to out_flat rows [205b+8, 205b+206) (also sequential).  The
  199th row of each tile is the next batch's token 0 (garbage); its
  out-write lands on the P_ row of b+1, which the final P_ DMA (issued
  last on the same ring) overwrites, and its contribution to the token
  sum is cancelled with a correction DMA of rows x[b+1, 0, :] (zero pad
  row for the last batch).
  Token sums accumulate batch-on-partition in PSUM via indicator-
  stationary matmuls (lhsT = e_b x ones), both u-halves into the same
  bank.  Tiny stage 2/3 computes aq and P_.
Host combines the small pool tensors:
    attnkT[d,k] = att[k,d] * nK[k,d],  attn2T[d,k] = att[k,d]^2,
    prflat[k,:] = prompt[k].reshape(6144)
aq is scale-invariant in x_mean, so the 1/197 mean scaling cancels and
the kernel works with raw token sums.
"""

import numpy as np

TOP_K = 10
LENGTH = 8
EMBED_DIM = 768
N_TOK = 197
B_FULL = 256
N_CORES = 8
B = B_FULL // N_CORES          # 32 batches per core
PF = LENGTH * EMBED_DIM        # 6144 flattened prompt row
XROWS = B * N_TOK + 1          # flat x rows incl one zero pad row
OROWS = B * (LENGTH + N_TOK) + 1   # flat out rows incl pad
NP2 = (N_TOK + 1) // 2         # 99 token pairs per batch (last half garbage)

_PROGRAMS = {}


def _build_program(K):
    import concourse.bacc as bacc
    import concourse.mybir as mybir
    import concourse.tile as tile
    from concourse.bass import ts
    from concourse.masks import make_identity

    f32 = mybir.dt.float32
    nc = bacc.Bacc()

    x = nc.dram_tensor("x", [XROWS, EMBED_DIM], f32, kind="ExternalInput")
    prflat = nc.dram_tensor("prflat", [K, PF], f32, kind="ExternalInput")
    attnkT = nc.dram_tensor("attnkT", [EMBED_DIM, K], f32, kind="ExternalInput")
    attn2T = nc.dram_tensor("attn2T", [EMBED_DIM, K], f32, kind="ExternalInput")
    emat = nc.dram_tensor("emat", [128, B, B], f32, kind="ExternalInput")
    out = nc.dram_tensor("out", [OROWS, EMBED_DIM], f32, kind="ExternalOutput")

    with tile.TileContext(nc) as tc:
        with (
            tc.tile_pool(name="const", bufs=1) as constp,
            tc.tile_pool(name="xt", bufs=8) as xtp,
            tc.tile_pool(name="xs", bufs=6) as xsp,
            tc.tile_pool(name="misc", bufs=1) as miscp,
            tc.tile_pool(name="psA", bufs=1, space="PSUM") as psap,
            tc.tile_pool(name="pst", bufs=1, space="PSUM") as pstp,
            tc.tile_pool(name="pp", bufs=2, space="PSUM") as ppp,
            tc.tile_pool(name="pt", bufs=2, space="PSUM") as ptp,
        ):
            # --- constants (gpsimd queue; big streams go on sync/scalar) ---
            ident = constp.tile([128, 128], f32)
            make_identity(nc, ident)
            prflat_sb = constp.tile([K, PF], f32)
            nc.gpsimd.dma_start(out=prflat_sb, in_=prflat[:, :])
            attnkT_sb = constp.tile([128, 6, K], f32)
            nc.gpsimd.dma_start(
                out=attnkT_sb,
                in_=attnkT[:, :].rearrange("(c p) k -> p c k", p=128))
            attn2T_sb = constp.tile([128, 6, K], f32)
            nc.gpsimd.dma_start(
                out=attn2T_sb,
                in_=attn2T[:, :].rearrange("(c p) k -> p c k", p=128))
            emat_sb = constp.tile([128, B, B], f32)
            nc.gpsimd.dma_start(out=emat_sb, in_=emat[:, :, :])
            # correction rows: x[b+1, token 0] for each b (pad row = 0 last)
            corr_sb = constp.tile([B, EMBED_DIM], f32)
            import concourse.bass as bass
            corr_ap = bass.AP(tensor=x[:, :].tensor, offset=N_TOK * EMBED_DIM,
                              ap=[[N_TOK * EMBED_DIM, B], [1, EMBED_DIM]])
            nc.gpsimd.dma_start(out=corr_sb, in_=corr_ap)

            # Preheat: have PE consume each constant once so no later matmul
            # needs >1 semaphore wait.
            scr = ptp.tile([1, 1], f32, tag="pt", name="scr")
            nc.tensor.matmul(scr, ident[:1, :1], ident[:1, :1],
                             start=True, stop=True)
            nc.tensor.matmul(scr, attnkT_sb[:1, 0, :1], attnkT_sb[:1, 0, :1],
                             start=True, stop=True)
            nc.tensor.matmul(scr, attn2T_sb[:1, 0, :1], attn2T_sb[:1, 0, :1],
                             start=True, stop=True)
            nc.tensor.matmul(scr, prflat_sb[:1, :1], prflat_sb[:1, :1],
                             start=True, stop=True)
            nc.tensor.matmul(scr, emat_sb[:1, 0, :1], emat_sb[:1, 0, :1],
                             start=True, stop=True)

            # token sums (+garbage), batch-on-partition, 2 psum halves
            psum_h = [psap.tile([B, 384], f32, tag=f"psum{h}", name=f"psum{h}")
                      for h in range(2)]

            # Byte-balance the three DMA queues (sync/scalar HWDGE ~1.0 rel
            # rate, gpsimd SWDGE ~0.56): in-DMAs on sync with a few spilled
            # to scalar; out-DMAs on scalar with some on gpsimd.
            in_eng = [nc.sync] * B
            for b in range(4, B, 4):
                if sum(1 for e in in_eng if e is nc.scalar) < 7:
                    in_eng[b] = nc.scalar
            out_eng = [nc.scalar] * B
            for b in range(1, B, 2):
                if sum(1 for e in out_eng if e is nc.gpsimd) < 14:
                    out_eng[b] = nc.gpsimd

            # --- stage 1: stream x, copy to out rows, accumulate sums ------
            for b in range(B):
                r0 = b * N_TOK
                o0 = b * (LENGTH + N_TOK) + LENGTH
                xt = xtp.tile([NP2, 2, EMBED_DIM], f32)
                in_eng[b].dma_start(
                    out=xt,
                    in_=x[r0:r0 + 2 * NP2, :].rearrange("(p u) d -> p u d", u=2))
                eng = out_eng[b]
                eng.dma_start(
                    out=out[o0:o0 + 2 * NP2, :].rearrange("(p u) d -> p u d",
                                                          u=2),
                    in_=xt)
                # fold the token pairs on DVE: halves the PE streaming volume
                xs = xsp.tile([NP2, EMBED_DIM], f32)
                nc.vector.tensor_add(xs, xt[:, 0, :], xt[:, 1, :])
                for h in range(2):
                    nc.tensor.matmul(
                        psum_h[h],
                        emat_sb[:NP2, b, :], xs[:, ts(h, 384)],
                        start=(b == 0), stop=(b == B - 1))

            # --- stage 2: subtract garbage, transpose, numer/norm2, aq -----
            means = miscp.tile([B, EMBED_DIM], f32)
            for h in range(2):
                nc.vector.tensor_sub(means[:, ts(h, 384)], psum_h[h],
                                     corr_sb[:, ts(h, 384)])

            meansT = miscp.tile([128, 6, B], f32)
            for j in range(6):
                pt = ptp.tile([128, B], f32)
                nc.tensor.transpose(pt, means[:, ts(j, 128)], ident[:B, :B])
                nc.vector.tensor_copy(meansT[:, j, :], pt)
            sqT = miscp.tile([128, 6, B], f32)
            nc.vector.tensor_mul(sqT, meansT, meansT)

            pn = pstp.tile([K, B], f32)
            pq = pstp.tile([K, B], f32)
            for j in range(6):
                nc.tensor.matmul(pn, attnkT_sb[:, j, :], meansT[:, j, :],
                                 start=(j == 0), stop=(j == 5))
            for j in range(6):
                nc.tensor.matmul(pq, attn2T_sb[:, j, :], sqT[:, j, :],
                                 start=(j == 0), stop=(j == 5))

            denom = miscp.tile([K, B], f32)
            nc.scalar.sqrt(denom, pq)
            nc.vector.tensor_scalar_max(denom, denom, 1e-12)
            recip = miscp.tile([K, B], f32)
            nc.vector.reciprocal(recip, denom)
            aqT = miscp.tile([K, B], f32)
            nc.vector.tensor_mul(aqT, pn, recip)

            # --- stage 3: P_ = aq @ prflat, write out P_ rows last ---------
            p_sb = miscp.tile([B, PF], f32)
            for h in range(PF // 384):
                pp = ppp.tile([B, 384], f32)
                nc.tensor.matmul(pp, aqT, prflat_sb[:, ts(h, 384)],
                                 start=True, stop=True)
                nc.vector.tensor_copy(p_sb[:, ts(h, 384)], pp)
            nc.scalar.dma_start(
                out=out[0:B * (LENGTH + N_TOK), :].rearrange(
                    "(b r) d -> b r d", r=LENGTH + N_TOK)[:, 0:LENGTH, :],
                in_=p_sb.rearrange("p (l d) -> p l d", l=LENGTH))

    nc.finalize()
    return nc


def _host_prep(prompt, attention, prompt_key, task_count):
    K = (int(task_count) + 1) * TOP_K
    pk = np.asarray(prompt_key[:K], dtype=np.float32)
    att = np.asarray(attention[:K], dtype=np.float32)
    pr = np.asarray(prompt[:K], dtype=np.float32)
    nrm = np.sqrt(np.sum(pk * pk, axis=1, keepdims=True, dtype=np.float32))
    nK = pk / np.maximum(nrm, np.float32(1e-12))
    attnkT = np.ascontiguousarray((att * nK).T)
    attn2T = np.ascontiguousarray((att * att).T)
    prflat = np.ascontiguousarray(pr.reshape(K, PF))
    return K, attnkT, attn2T, prflat


def _make_emat():
    emat = np.zeros((128, B, B), dtype=np.float32)
    for b in range(B):
        emat[:, b, b] = 1.0
    return emat


def _shard_x(x_embed, i):
    flat = x_embed[i * B:(i + 1) * B].reshape(B * N_TOK, EMBED_DIM)
    pad = np.zeros((1, EMBED_DIM), dtype=np.float32)
    return np.ascontiguousarray(np.concatenate([flat, pad], axis=0))


def kernel(x_embed, prompt, attention, prompt_key, iseval, task_count,
           _want_trace=False, **_trace_kwargs):
    from concourse.bass_utils import run_bass_kernel_spmd

    x_embed = np.asarray(x_embed, dtype=np.float32)
    assert x_embed.shape == (B_FULL, N_TOK, EMBED_DIM)
    K, attnkT, attn2T, prflat = _host_prep(prompt, attention, prompt_key,
                                           task_count)

    if K not in _PROGRAMS:
        _PROGRAMS[K] = _build_program(K)
    nc = _PROGRAMS[K]

    emat = _make_emat()
    in_maps = []
    for i in range(N_CORES):
        in_maps.append({
            "x": _shard_x(x_embed, i),
            "prflat": prflat,
            "attnkT": attnkT,
            "attn2T": attn2T,
            "emat": emat,
        })
    res = run_bass_kernel_spmd(nc, in_maps, core_ids=list(range(N_CORES)),
                               trace=_want_trace, **_trace_kwargs)
    full = np.concatenate(
        [res.results[i]["out"][:B * (LENGTH + N_TOK)].reshape(
            B, LENGTH + N_TOK, EMBED_DIM) for i in range(N_CORES)],
        axis=0)
    if _want_trace:
        return full, res
    return full



# revision 37
# speedup vs baseline: 1.0658x; 1.0391x over previous
"""CODA-Prompt forward kernel for 8 TRN2 NeuronCores (data-parallel over batch).

Reference computation (forward only; stop_gradient is identity):
    K = (task_count + 1) * 10            # active pool slice, all branches
    x_mean[b,d]  = mean_n x[b,n,d]
    aq[b,k]      = (x_mean . (att[k]*nK[k])) / max(||x_mean*att[k]||, eps)
    P_[b,l,d]    = sum_k aq[b,k] * prompt[k,l,d]
    out          = concat([P_, x], axis=1)            # [B, 8+197, 768]

Device kernel per core (B=32 of 256 batches), built for DMA efficiency:
  x arrives flat+padded [B*197+1, 768].  Each batch is one fully
  sequential DMA in token-pair layout [99, 2, 768] (6 KB runs), copied
  back out to out_flat rows [205b+8, 205b+206) (also sequential).  The
  199th row of each tile is the next batch's token 0 (garbage); its
  out-write lands on the P_ row of b+1, which the final P_ DMA (issued
  last on the same ring) overwrites, and its contribution to the token
  sum is cancelled with a correction DMA of rows x[b+1, 0, :] (zero pad
  row for the last batch).
  Token sums accumulate batch-on-partition in PSUM via indicator-
  stationary matmuls (lhsT = e_b x ones), both u-halves into the same
  bank.  Tiny stage 2/3 computes aq and P_.
Host combines the small pool tensors:
    attnkT[d,k] = att[k,d] * nK[k,d],  attn2T[d,k] = att[k,d]^2,
    prflat[k,:] = prompt[k].reshape(6144)
aq is scale-invariant in x_mean, so the 1/197 mean scaling cancels and
the kernel works with raw token sums.
"""

import numpy as np

TOP_K = 10
LENGTH = 8
EMBED_DIM = 768
N_TOK = 197
B_FULL = 256
N_CORES = 8
B = B_FULL // N_CORES          # 32 batches per core
PF = LENGTH * EMBED_DIM        # 6144 flattened prompt row
XROWS = B * N_TOK + 1          # flat x rows incl one zero pad row
OROWS = B * (LENGTH + N_TOK) + 1   # flat out rows incl pad
NP2 = (N_TOK + 1) // 2         # 99 token pairs per batch (last half garbage)

_PROGRAMS = {}


def _build_program(K):
    import concourse.bacc as bacc
    import concourse.mybir as mybir
    import concourse.tile as tile
    from concourse.bass import ts
    from concourse.masks import make_identity

    f32 = mybir.dt.float32
    nc = bacc.Bacc()

    x = nc.dram_tensor("x", [XROWS, EMBED_DIM], f32, kind="ExternalInput")
    prflat = nc.dram_tensor("prflat", [K, PF], f32, kind="ExternalInput")
    attnkT = nc.dram_tensor("attnkT", [EMBED_DIM, K], f32, kind="ExternalInput")
    attn2T = nc.dram_tensor("attn2T", [EMBED_DIM, K], f32, kind="ExternalInput")
    emat = nc.dram_tensor("emat", [128, B, B], f32, kind="ExternalInput")
    out = nc.dram_tensor("out", [OROWS, EMBED_DIM], f32, kind="ExternalOutput")

    with tile.TileContext(nc) as tc:
        with (
            tc.tile_pool(name="const", bufs=1) as constp,
            tc.tile_pool(name="xt", bufs=8) as xtp,
            tc.tile_pool(name="xs", bufs=6) as xsp,
            tc.tile_pool(name="misc", bufs=1) as miscp,
            tc.tile_pool(name="psA", bufs=1, space="PSUM") as psap,
            tc.tile_pool(name="pst", bufs=1, space="PSUM") as pstp,
            tc.tile_pool(name="pp", bufs=2, space="PSUM") as ppp,
            tc.tile_pool(name="pt", bufs=2, space="PSUM") as ptp,
        ):
            # --- constants (gpsimd queue; big streams go on sync/scalar) ---
            ident = constp.tile([128, 128], f32)
            make_identity(nc, ident)
            # prflat lands on partitions 96+ (idle DMA engines), then one
            # DVE copy moves it to base 0 for the PE (SBUF->SBUF costs no
            # DMA-engine time; engines 0-3 are the window-critical ones)
            prflat_hi = constp.tile([128, PF], f32)
            nc.gpsimd.dma_start(out=prflat_hi[96:96 + K], in_=prflat[:, :])
            prflat_sb = constp.tile([K, PF], f32)
            nc.vector.tensor_copy(prflat_sb, prflat_hi[96:96 + K, :])
            attnkT_sb = constp.tile([128, 6, K], f32)
            nc.gpsimd.dma_start(
                out=attnkT_sb,
                in_=attnkT[:, :].rearrange("(c p) k -> p c k", p=128))
            attn2T_sb = constp.tile([128, 6, K], f32)
            nc.gpsimd.dma_start(
                out=attn2T_sb,
                in_=attn2T[:, :].rearrange("(c p) k -> p c k", p=128))
            emat_sb = constp.tile([128, B, B], f32)
            nc.gpsimd.dma_start(out=emat_sb, in_=emat[:, :, :])
            # correction rows: x[b+1, token 0] for each b (pad row = 0 last)
            corr_hi = constp.tile([128, EMBED_DIM], f32)
            import concourse.bass as bass
            corr_ap = bass.AP(tensor=x[:, :].tensor, offset=N_TOK * EMBED_DIM,
                              ap=[[N_TOK * EMBED_DIM, B], [1, EMBED_DIM]])
            nc.gpsimd.dma_start(out=corr_hi[96:96 + B], in_=corr_ap)

            # Preheat: have PE consume each constant once so no later matmul
            # needs >1 semaphore wait.
            scr = ptp.tile([1, 1], f32, tag="pt", name="scr")
            nc.tensor.matmul(scr, ident[:1, :1], ident[:1, :1],
                             start=True, stop=True)
            nc.tensor.matmul(scr, attnkT_sb[:1, 0, :1], attnkT_sb[:1, 0, :1],
                             start=True, stop=True)
            nc.tensor.matmul(scr, attn2T_sb[:1, 0, :1], attn2T_sb[:1, 0, :1],
                             start=True, stop=True)
            nc.tensor.matmul(scr, prflat_sb[:1, :1], prflat_sb[:1, :1],
                             start=True, stop=True)
            nc.tensor.matmul(scr, emat_sb[:1, 0, :1], emat_sb[:1, 0, :1],
                             start=True, stop=True)

            # token sums (+garbage), batch-on-partition, 2 psum halves
            psum_h = [psap.tile([B, 384], f32, tag=f"psum{h}", name=f"psum{h}")
                      for h in range(2)]

            # Byte-balance the three DMA queues (sync/scalar HWDGE ~1.0 rel
            # rate, gpsimd SWDGE ~0.56): in-DMAs on sync with a few spilled
            # to scalar; out-DMAs on scalar with some on gpsimd.
            in_eng = [nc.sync] * B
            for b in range(4, B, 4):
                if sum(1 for e in in_eng if e is nc.scalar) < 7:
                    in_eng[b] = nc.scalar
            out_eng = [nc.scalar] * B
            for b in range(1, B, 2):
                if sum(1 for e in out_eng if e is nc.gpsimd) < 14:
                    out_eng[b] = nc.gpsimd

            # every batch's odd 197th row in one strided DRAM->DRAM DMA
            # (SBUF APs cannot start at partition 98); rows are untouched
            # by anything else, so it can fire immediately.
            orow = (LENGTH + N_TOK) * EMBED_DIM
            nc.gpsimd.dma_start(
                out=bass.AP(tensor=out[:, :].tensor,
                            offset=(LENGTH + N_TOK - 1) * EMBED_DIM,
                            ap=[[orow, B], [1, EMBED_DIM]]),
                in_=bass.AP(tensor=x[:, :].tensor,
                            offset=(N_TOK - 1) * EMBED_DIM,
                            ap=[[N_TOK * EMBED_DIM, B], [1, EMBED_DIM]]))

            # --- stage 1: stream x, copy to out rows, accumulate sums ------
            for b in range(B):
                r0 = b * N_TOK
                o0 = b * (LENGTH + N_TOK) + LENGTH
                xt = xtp.tile([NP2, 2, EMBED_DIM], f32)
                in_eng[b].dma_start(
                    out=xt,
                    in_=x[r0:r0 + 2 * NP2, :].rearrange("(p u) d -> p u d", u=2))
                eng = out_eng[b]
                eng.dma_start(
                    out=out[o0:o0 + N_TOK - 1, :].rearrange(
                        "(p u) d -> p u d", u=2),
                    in_=xt[:NP2 - 1])
                # fold the token pairs on DVE: halves the PE streaming volume
                xs = xsp.tile([NP2, EMBED_DIM], f32)
                nc.vector.tensor_add(xs, xt[:, 0, :], xt[:, 1, :])
                for h in range(2):
                    nc.tensor.matmul(
                        psum_h[h],
                        emat_sb[:NP2, b, :], xs[:, ts(h, 384)],
                        start=(b == 0), stop=(b == B - 1))

            # --- stage 2: subtract garbage, transpose, numer/norm2, aq -----
            means = miscp.tile([B, EMBED_DIM], f32)
            for h in range(2):
                nc.vector.tensor_sub(means[:, ts(h, 384)], psum_h[h],
                                     corr_hi[96:96 + B, ts(h, 384)])

            meansT = miscp.tile([128, 6, B], f32)
            for j in range(6):
                pt = ptp.tile([128, B], f32)
                nc.tensor.transpose(pt, means[:, ts(j, 128)], ident[:B, :B])
                nc.vector.tensor_copy(meansT[:, j, :], pt)
            sqT = miscp.tile([128, 6, B], f32)
            nc.vector.tensor_mul(sqT, meansT, meansT)

            pn = pstp.tile([K, B], f32)
            pq = pstp.tile([K, B], f32)
            for j in range(6):
                nc.tensor.matmul(pn, attnkT_sb[:, j, :], meansT[:, j, :],
                                 start=(j == 0), stop=(j == 5))
            for j in range(6):
                nc.tensor.matmul(pq, attn2T_sb[:, j, :], sqT[:, j, :],
                                 start=(j == 0), stop=(j == 5))

            denom = miscp.tile([K, B], f32)
            nc.scalar.sqrt(denom, pq)
            nc.vector.tensor_scalar_max(denom, denom, 1e-12)
            recip = miscp.tile([K, B], f32)
            nc.vector.reciprocal(recip, denom)
            aqT = miscp.tile([K, B], f32)
            nc.vector.tensor_mul(aqT, pn, recip)

            # --- stage 3: P_ = aq @ prflat, strips pipelined onto the ----
            # (empty by now) sync ring; p_sb at partitions 96+ so the P_
            # readback also rides the idle DMA engines
            p_sb = miscp.tile([128, PF], f32)
            for h in range(PF // 384):
                pp = ppp.tile([B, 384], f32)
                nc.tensor.matmul(pp, aqT, prflat_sb[:, ts(h, 384)],
                                 start=True, stop=True)
                if h % 2 == 0:
                    nc.vector.tensor_copy(p_sb[96:96 + B, ts(h, 384)], pp)
                else:
                    nc.scalar.copy(p_sb[96:96 + B, ts(h, 384)], pp)
                if h % 4 == 3:  # 1536 cols = 2 whole P_ rows per strip
                    c0 = (h - 3) * 384
                    ls = c0 // EMBED_DIM
                    nc.sync.dma_start(
                        out=out[0:B * (LENGTH + N_TOK), :].rearrange(
                            "(b r) d -> b r d",
                            r=LENGTH + N_TOK)[:, ls:ls + 2, :],
                        in_=p_sb[96:96 + B, c0:c0 + 1536].rearrange(
                            "p (l d) -> p l d", l=2))

    nc.finalize()
    return nc


def _host_prep(prompt, attention, prompt_key, task_count):
    K = (int(task_count) + 1) * TOP_K
    pk = np.asarray(prompt_key[:K], dtype=np.float32)
    att = np.asarray(attention[:K], dtype=np.float32)
    pr = np.asarray(prompt[:K], dtype=np.float32)
    nrm = np.sqrt(np.sum(pk * pk, axis=1, keepdims=True, dtype=np.float32))
    nK = pk / np.maximum(nrm, np.float32(1e-12))
    attnkT = np.ascontiguousarray((att * nK).T)
    attn2T = np.ascontiguousarray((att * att).T)
    prflat = np.ascontiguousarray(pr.reshape(K, PF))
    return K, attnkT, attn2T, prflat


def _make_emat():
    emat = np.zeros((128, B, B), dtype=np.float32)
    for b in range(B):
        emat[:, b, b] = 1.0
    return emat


def _shard_x(x_embed, i):
    flat = x_embed[i * B:(i + 1) * B].reshape(B * N_TOK, EMBED_DIM)
    pad = np.zeros((1, EMBED_DIM), dtype=np.float32)
    return np.ascontiguousarray(np.concatenate([flat, pad], axis=0))


def kernel(x_embed, prompt, attention, prompt_key, iseval, task_count,
           _want_trace=False, **_trace_kwargs):
    from concourse.bass_utils import run_bass_kernel_spmd

    x_embed = np.asarray(x_embed, dtype=np.float32)
    assert x_embed.shape == (B_FULL, N_TOK, EMBED_DIM)
    K, attnkT, attn2T, prflat = _host_prep(prompt, attention, prompt_key,
                                           task_count)

    if K not in _PROGRAMS:
        _PROGRAMS[K] = _build_program(K)
    nc = _PROGRAMS[K]

    emat = _make_emat()
    in_maps = []
    for i in range(N_CORES):
        in_maps.append({
            "x": _shard_x(x_embed, i),
            "prflat": prflat,
            "attnkT": attnkT,
            "attn2T": attn2T,
            "emat": emat,
        })
    res = run_bass_kernel_spmd(nc, in_maps, core_ids=list(range(N_CORES)),
                               trace=_want_trace, **_trace_kwargs)
    full = np.concatenate(
        [res.results[i]["out"][:B * (LENGTH + N_TOK)].reshape(
            B, LENGTH + N_TOK, EMBED_DIM) for i in range(N_CORES)],
        axis=0)
    if _want_trace:
        return full, res
    return full



# revision 38
# speedup vs baseline: 1.0771x; 1.0106x over previous
"""CODA-Prompt forward kernel for 8 TRN2 NeuronCores (data-parallel over batch).

Reference computation (forward only; stop_gradient is identity):
    K = (task_count + 1) * 10            # active pool slice, all branches
    x_mean[b,d]  = mean_n x[b,n,d]
    aq[b,k]      = (x_mean . (att[k]*nK[k])) / max(||x_mean*att[k]||, eps)
    P_[b,l,d]    = sum_k aq[b,k] * prompt[k,l,d]
    out          = concat([P_, x], axis=1)            # [B, 8+197, 768]

Device kernel per core (B=32 of 256 batches), built for DMA efficiency:
  x arrives flat+padded [B*197+1, 768].  Each batch is one fully
  sequential DMA in token-pair layout [99, 2, 768] (6 KB runs), copied
  back out to out_flat rows [205b+8, 205b+206) (also sequential).  The
  199th row of each tile is the next batch's token 0 (garbage); its
  out-write lands on the P_ row of b+1, which the final P_ DMA (issued
  last on the same ring) overwrites, and its contribution to the token
  sum is cancelled with a correction DMA of rows x[b+1, 0, :] (zero pad
  row for the last batch).
  Token sums accumulate batch-on-partition in PSUM via indicator-
  stationary matmuls (lhsT = e_b x ones), both u-halves into the same
  bank.  Tiny stage 2/3 computes aq and P_.
Host combines the small pool tensors:
    attnkT[d,k] = att[k,d] * nK[k,d],  attn2T[d,k] = att[k,d]^2,
    prflat[k,:] = prompt[k].reshape(6144)
aq is scale-invariant in x_mean, so the 1/197 mean scaling cancels and
the kernel works with raw token sums.
"""

import numpy as np

TOP_K = 10
LENGTH = 8
EMBED_DIM = 768
N_TOK = 197
B_FULL = 256
N_CORES = 8
B = B_FULL // N_CORES          # 32 batches per core
PF = LENGTH * EMBED_DIM        # 6144 flattened prompt row
XROWS = B * N_TOK + 1          # flat x rows incl one zero pad row
OROWS = B * (LENGTH + N_TOK) + 1   # flat out rows incl pad
NP2 = (N_TOK + 1) // 2         # 99 token pairs per batch (last half garbage)

_PROGRAMS = {}


def _build_program(K):
    import concourse.bacc as bacc
    import concourse.mybir as mybir
    import concourse.tile as tile
    from concourse.bass import ts
    from concourse.masks import make_identity

    f32 = mybir.dt.float32
    nc = bacc.Bacc()

    x = nc.dram_tensor("x", [XROWS, EMBED_DIM], f32, kind="ExternalInput")
    prflat = nc.dram_tensor("prflat", [K, PF], f32, kind="ExternalInput")
    attnkT = nc.dram_tensor("attnkT", [EMBED_DIM, K], f32, kind="ExternalInput")
    attn2T = nc.dram_tensor("attn2T", [EMBED_DIM, K], f32, kind="ExternalInput")
    emat = nc.dram_tensor("emat", [128, B, B], f32, kind="ExternalInput")
    out = nc.dram_tensor("out", [OROWS, EMBED_DIM], f32, kind="ExternalOutput")

    with tile.TileContext(nc) as tc:
        with (
            tc.tile_pool(name="const", bufs=1) as constp,
            tc.tile_pool(name="xt", bufs=10) as xtp,
            tc.tile_pool(name="xs", bufs=6) as xsp,
            tc.tile_pool(name="misc", bufs=1) as miscp,
            tc.tile_pool(name="psA", bufs=1, space="PSUM") as psap,
            tc.tile_pool(name="pst", bufs=1, space="PSUM") as pstp,
            tc.tile_pool(name="pp", bufs=2, space="PSUM") as ppp,
            tc.tile_pool(name="pt", bufs=2, space="PSUM") as ptp,
        ):
            # --- constants (gpsimd queue; big streams go on sync/scalar) ---
            ident = constp.tile([128, 128], f32)
            make_identity(nc, ident)
            # prflat lands on partitions 96+ (idle DMA engines), then one
            # DVE copy moves it to base 0 for the PE (SBUF->SBUF costs no
            # DMA-engine time; engines 0-3 are the window-critical ones)
            prflat_hi = constp.tile([128, PF], f32)
            nc.gpsimd.dma_start(out=prflat_hi[96:96 + K], in_=prflat[:, :])
            prflat_sb = constp.tile([K, PF], f32)
            nc.vector.tensor_copy(prflat_sb, prflat_hi[96:96 + K, :])
            attnkT_sb = constp.tile([128, 6, K], f32)
            nc.gpsimd.dma_start(
                out=attnkT_sb,
                in_=attnkT[:, :].rearrange("(c p) k -> p c k", p=128))
            attn2T_sb = constp.tile([128, 6, K], f32)
            nc.gpsimd.dma_start(
                out=attn2T_sb,
                in_=attn2T[:, :].rearrange("(c p) k -> p c k", p=128))
            emat_sb = constp.tile([128, B, B], f32)
            nc.gpsimd.dma_start(out=emat_sb, in_=emat[:, :, :])
            # correction rows: x[b+1, token 0] for each b (pad row = 0 last)
            corr_hi = constp.tile([128, EMBED_DIM], f32)
            import concourse.bass as bass
            corr_ap = bass.AP(tensor=x[:, :].tensor, offset=N_TOK * EMBED_DIM,
                              ap=[[N_TOK * EMBED_DIM, B], [1, EMBED_DIM]])
            nc.gpsimd.dma_start(out=corr_hi[96:96 + B], in_=corr_ap)

            # Preheat: have PE consume each constant once so no later matmul
            # needs >1 semaphore wait.
            scr = ptp.tile([1, 1], f32, tag="pt", name="scr")
            nc.tensor.matmul(scr, ident[:1, :1], ident[:1, :1],
                             start=True, stop=True)
            nc.tensor.matmul(scr, attnkT_sb[:1, 0, :1], attnkT_sb[:1, 0, :1],
                             start=True, stop=True)
            nc.tensor.matmul(scr, attn2T_sb[:1, 0, :1], attn2T_sb[:1, 0, :1],
                             start=True, stop=True)
            nc.tensor.matmul(scr, prflat_sb[:1, :1], prflat_sb[:1, :1],
                             start=True, stop=True)
            nc.tensor.matmul(scr, emat_sb[:1, 0, :1], emat_sb[:1, 0, :1],
                             start=True, stop=True)

            # token sums (+garbage), batch-on-partition, 2 psum halves
            psum_h = [psap.tile([B, 384], f32, tag=f"psum{h}", name=f"psum{h}")
                      for h in range(2)]

            # Byte-balance the three DMA queues (sync/scalar HWDGE ~1.0 rel
            # rate, gpsimd SWDGE ~0.56): in-DMAs on sync with a few spilled
            # to scalar; out-DMAs on scalar with some on gpsimd.
            in_eng = [nc.sync] * B
            for b in range(4, B, 4):
                if sum(1 for e in in_eng if e is nc.scalar) < 7:
                    in_eng[b] = nc.scalar
            out_eng = [nc.scalar] * B
            for b in range(1, B, 2):
                if sum(1 for e in out_eng if e is nc.gpsimd) < 14:
                    out_eng[b] = nc.gpsimd

            # every batch's odd 197th row in one strided DRAM->DRAM DMA
            # (SBUF APs cannot start at partition 98); rows are untouched
            # by anything else, so it can fire immediately.
            orow = (LENGTH + N_TOK) * EMBED_DIM
            nc.gpsimd.dma_start(
                out=bass.AP(tensor=out[:, :].tensor,
                            offset=(LENGTH + N_TOK - 1) * EMBED_DIM,
                            ap=[[orow, B], [1, EMBED_DIM]]),
                in_=bass.AP(tensor=x[:, :].tensor,
                            offset=(N_TOK - 1) * EMBED_DIM,
                            ap=[[N_TOK * EMBED_DIM, B], [1, EMBED_DIM]]))

            # --- stage 1: stream x, copy to out rows, accumulate sums ------
            for b in range(B):
                r0 = b * N_TOK
                o0 = b * (LENGTH + N_TOK) + LENGTH
                xt = xtp.tile([NP2, 2, EMBED_DIM], f32)
                in_eng[b].dma_start(
                    out=xt,
                    in_=x[r0:r0 + 2 * NP2, :].rearrange("(p u) d -> p u d", u=2))
                eng = out_eng[b]
                eng.dma_start(
                    out=out[o0:o0 + N_TOK - 1, :].rearrange(
                        "(p u) d -> p u d", u=2),
                    in_=xt[:NP2 - 1])
                # fold the token pairs on DVE: halves the PE streaming volume
                xs = xsp.tile([NP2, EMBED_DIM], f32)
                nc.vector.tensor_add(xs, xt[:, 0, :], xt[:, 1, :])
                for h in range(2):
                    nc.tensor.matmul(
                        psum_h[h],
                        emat_sb[:NP2, b, :], xs[:, ts(h, 384)],
                        start=(b == 0), stop=(b == B - 1))

            # PE warm-up: ~5us of back-to-back matmuls while stage 2's DVE
            # work runs -- releases the HAM clock-gate so stage 3's fp32
            # stream runs at full clock instead of the throttled rate.
            ptw = ptp.tile([128, B], f32, tag="pt", name="pt")
            for w in range(16):
                nc.tensor.matmul(ptw, ident, ident[:, :B],
                                 start=True, stop=True)

            # --- stage 2: subtract garbage, transpose, numer/norm2, aq -----
            means = miscp.tile([B, EMBED_DIM], f32)
            for h in range(2):
                nc.vector.tensor_sub(means[:, ts(h, 384)], psum_h[h],
                                     corr_hi[96:96 + B, ts(h, 384)])

            meansT = miscp.tile([128, 6, B], f32)
            for j in range(6):
                pt = ptp.tile([128, B], f32)
                nc.tensor.transpose(pt, means[:, ts(j, 128)], ident[:B, :B])
                nc.vector.tensor_copy(meansT[:, j, :], pt)
            sqT = miscp.tile([128, 6, B], f32)
            nc.vector.tensor_mul(sqT, meansT, meansT)

            pn = pstp.tile([K, B], f32)
            pq = pstp.tile([K, B], f32)
            for j in range(6):
                nc.tensor.matmul(pn, attnkT_sb[:, j, :], meansT[:, j, :],
                                 start=(j == 0), stop=(j == 5))
            for j in range(6):
                nc.tensor.matmul(pq, attn2T_sb[:, j, :], sqT[:, j, :],
                                 start=(j == 0), stop=(j == 5))

            denom = miscp.tile([K, B], f32)
            nc.scalar.sqrt(denom, pq)
            nc.vector.tensor_scalar_max(denom, denom, 1e-12)
            recip = miscp.tile([K, B], f32)
            nc.vector.reciprocal(recip, denom)
            aqT = miscp.tile([K, B], f32)
            nc.vector.tensor_mul(aqT, pn, recip)

            # --- stage 3: P_ = aq @ prflat, strips pipelined onto the ----
            # (empty by now) sync ring; p_sb at partitions 96+ so the P_
            # readback also rides the idle DMA engines
            p_sb = miscp.tile([128, PF], f32)
            for h in range(PF // 384):
                pp = ppp.tile([B, 384], f32)
                nc.tensor.matmul(pp, aqT, prflat_sb[:, ts(h, 384)],
                                 start=True, stop=True)
                if h % 2 == 0:
                    nc.vector.tensor_copy(p_sb[96:96 + B, ts(h, 384)], pp)
                else:
                    nc.scalar.copy(p_sb[96:96 + B, ts(h, 384)], pp)
                if h % 4 == 3:  # 1536 cols = 2 whole P_ rows per strip
                    c0 = (h - 3) * 384
                    ls = c0 // EMBED_DIM
                    nc.sync.dma_start(
                        out=out[0:B * (LENGTH + N_TOK), :].rearrange(
                            "(b r) d -> b r d",
                            r=LENGTH + N_TOK)[:, ls:ls + 2, :],
                        in_=p_sb[96:96 + B, c0:c0 + 1536].rearrange(
                            "p (l d) -> p l d", l=2))

    nc.finalize()
    return nc


def _host_prep(prompt, attention, prompt_key, task_count):
    K = (int(task_count) + 1) * TOP_K
    pk = np.asarray(prompt_key[:K], dtype=np.float32)
    att = np.asarray(attention[:K], dtype=np.float32)
    pr = np.asarray(prompt[:K], dtype=np.float32)
    nrm = np.sqrt(np.sum(pk * pk, axis=1, keepdims=True, dtype=np.float32))
    nK = pk / np.maximum(nrm, np.float32(1e-12))
    attnkT = np.ascontiguousarray((att * nK).T)
    attn2T = np.ascontiguousarray((att * att).T)
    prflat = np.ascontiguousarray(pr.reshape(K, PF))
    return K, attnkT, attn2T, prflat


def _make_emat():
    emat = np.zeros((128, B, B), dtype=np.float32)
    for b in range(B):
        emat[:, b, b] = 1.0
    return emat


def _shard_x(x_embed, i):
    flat = x_embed[i * B:(i + 1) * B].reshape(B * N_TOK, EMBED_DIM)
    pad = np.zeros((1, EMBED_DIM), dtype=np.float32)
    return np.ascontiguousarray(np.concatenate([flat, pad], axis=0))


def kernel(x_embed, prompt, attention, prompt_key, iseval, task_count,
           _want_trace=False, **_trace_kwargs):
    from concourse.bass_utils import run_bass_kernel_spmd

    x_embed = np.asarray(x_embed, dtype=np.float32)
    assert x_embed.shape == (B_FULL, N_TOK, EMBED_DIM)
    K, attnkT, attn2T, prflat = _host_prep(prompt, attention, prompt_key,
                                           task_count)

    if K not in _PROGRAMS:
        _PROGRAMS[K] = _build_program(K)
    nc = _PROGRAMS[K]

    emat = _make_emat()
    in_maps = []
    for i in range(N_CORES):
        in_maps.append({
            "x": _shard_x(x_embed, i),
            "prflat": prflat,
            "attnkT": attnkT,
            "attn2T": attn2T,
            "emat": emat,
        })
    res = run_bass_kernel_spmd(nc, in_maps, core_ids=list(range(N_CORES)),
                               trace=_want_trace, **_trace_kwargs)
    full = np.concatenate(
        [res.results[i]["out"][:B * (LENGTH + N_TOK)].reshape(
            B, LENGTH + N_TOK, EMBED_DIM) for i in range(N_CORES)],
        axis=0)
    if _want_trace:
        return full, res
    return full

